# revision 10
# baseline (speedup 1.0000x reference)
"""Bass/Trainium2 kernel for nn_EnhancedBianGuaAttention_76055280878201.

Contract: kernel(**inputs) takes the FULL unsharded inputs (as produced by
reference.setup_inputs()) and returns the FULL (B, T, D) output.

Sharding: 8 cores = 2 batches x 4 head-groups (4 heads each).  Each core:
  - computes q/k/v projections (transposed layout) for its 4 heads from x[b]^T
  - computes u^T = tanh([hex_w; q6_w] @ x^T)  (12, T) and per-head
    A_h^T = B_h^T u^T where B_h = diag(lam/2 * I6, sig(scale)*2/6 * Mh)
    so that the full score bias is bias[i,j] = sum_c u[j,c] A_h[i,c]
  - flash-style causal attention, scores computed transposed (keys on the
    partition axis):  E^T[j,i] = exp(alpha*tanh(qk/beta) + bias^T),
    masked with affine_select; denominators via a ones-column appended to V
  - its 4 heads' slice of the output projection, written transposed
Host side: sums the 4 partial outputs per batch and transposes.

Precision: the projection/score/AV chains run in SC_DT (bf16 by default; PE
full rate, cheap ACT/DVE, half DMA).  The output projection and the softmax
normalization stay in fp32r/fp32 (PSUM accumulation is always fp32).
"""

import os
import sys

import numpy as np

for _p in ("/opt/trn_rl_repo", "/root/.axon_site/_ro/trn_rl_repo"):
    if os.path.isdir(_p) and _p not in sys.path:
        sys.path.append(_p)

import ml_dtypes
import concourse.bacc as bacc
import concourse.mybir as mybir
import concourse.tile as tile
from concourse.bass_utils import run_bass_kernel_spmd

B, T, D, H, NT = 2, 2048, 1024, 16, 7
HD = D // H          # 64
TEMP = 0.5
NCORES = 8
HPC = 4              # heads per core
CPB = NCORES // B    # cores per batch (4)
TC = 512             # query-chunk size
NTC = T // TC        # 4
JB = 128             # key-block size
NJB = T // JB        # 16
KC = D // 128        # contraction chunks for the projections (8)

F32 = mybir.dt.float32
F32R = mybir.dt.float32r
BF16 = mybir.dt.bfloat16
Act = mybir.ActivationFunctionType
Alu = mybir.AluOpType

# score-chain dtype: BF16 (fast) or F32R (precise)
USE_BF16 = os.environ.get("KERNEL_F32R") != "1"


def _emit(nc, tc_, dr, alpha, inv_beta):
    """Emit the per-core program. dr: dict of DRAM APs."""
    SC = BF16 if USE_BF16 else F32R
    xT_r = dr["xT"].rearrange("(c p) t -> c p t", p=128)       # (8,128,T)
    wqkv_r = dr["wqkvT"].rearrange("(c p) m -> c p m", p=128)  # (8,128,768)
    whq_r = dr["whqT"].rearrange("(c p) w -> p c w", p=128)    # (128,8,12)

    with (
        tc_.tile_pool(name="persist", bufs=1) as pp,
        tc_.tile_pool(name="work", bufs=1) as wp,
        tc_.tile_pool(name="psum", bufs=1, space="PSUM") as sp,
    ):
        # ---- constants / weights (pre-rounded on host) --------------
        ident = pp.tile([128, 128], SC)
        nc.sync.dma_start(out=ident[:], in_=dr["ident"])
        ones64 = pp.tile([1, 64], F32R)
        nc.sync.dma_start(out=ones64[:], in_=dr["ones64"])
        bTt = pp.tile([12, 12 * HPC], SC)
        nc.sync.dma_start(out=bTt[:], in_=dr["bT"])
        owt = [pp.tile([128, D], F32R, name=f"owt{i}") for i in range(2)]
        for i in range(2):
            nc.sync.dma_start(out=owt[i][:], in_=dr["owT"][i * 128:(i + 1) * 128, :])
        whq = wp.tile([128, KC, 12], SC)
        nc.sync.dma_start(out=whq[:], in_=whq_r)
        wq = [wp.tile([128, 3 * HPC * HD], SC, name=f"wq{c}") for c in range(KC)]
        for c in range(KC):
            nc.sync.dma_start(out=wq[c][:], in_=wqkv_r[c])
        ones_col = pp.tile([128, 1], F32)
        nc.gpsimd.memset(ones_col[:], 1.0)

        # ---- persistent activations ---------------------------------
        uT = pp.tile([12, T], SC)
        # q/k transposed, 2 heads per tile: rows h%2*64 .. +64
        qt = [pp.tile([128, T], SC, name=f"qt{i}") for i in range(2)]
        kt = [pp.tile([128, T], SC, name=f"kt{i}") for i in range(2)]
        vt = [pp.tile([128, T], SC, name=f"vt{i}") for i in range(2)]
        qkv_tiles = qt + kt + vt  # oc order: q01,q23,k01,k23,v01,v23
        # V' per head: natural layout + ones column, 16 blocks of (128, 65)
        vp = [pp.tile([128, NJB * (HD + 1)], SC, name=f"vp{h}")
              for h in range(HPC)]
        # normalized attention out (transposed, f32r), 2 heads per tile
        ao = [pp.tile([128, T], F32R, name=f"ao{i}") for i in range(2)]

        # ---- phase A: projections + u, streamed over t-chunks -------
        for t4 in range(NTC):
            sl = slice(t4 * TC, (t4 + 1) * TC)
            xt = [
                wp.tile([128, TC], SC, name=f"xt{c}", tag=f"xt{c}", bufs=2)
                for c in range(KC)
            ]
            for c in range(KC):
                nc.sync.dma_start(out=xt[c][:], in_=xT_r[c, :, sl])

            # u^T chunk
            pu = sp.tile([12, TC], F32, name="pu", tag="mm", bufs=4)
            for c in range(KC):
                nc.tensor.matmul(pu[:], whq[:, c, :], xt[c][:],
                                 start=(c == 0), stop=(c == KC - 1))
            nc.scalar.activation(uT[:, sl], pu[:], Act.Tanh)

            # qkv projections
            for oc in range(6):
                pq = sp.tile([128, TC], F32, name="pq", tag="mm", bufs=4)
                for c in range(KC):
                    nc.tensor.matmul(pq[:], wq[c][:, oc * 128:(oc + 1) * 128],
                                     xt[c][:], start=(c == 0), stop=(c == KC - 1))
                nc.vector.tensor_copy(qkv_tiles[oc][:, sl], pq[:])

        # ---- phase B: build V' (natural layout + ones col) ----------
        for h in range(HPC):
            ro = (h % 2) * HD
            for tb in range(NJB):
                pv = sp.tile([128, HD], SC, name="pv", tag="aux", bufs=2)
                nc.tensor.transpose(
                    pv[:], vt[h // 2][ro:ro + HD, tb * JB:(tb + 1) * JB],
                    ident[ro:ro + HD, ro:ro + HD])
                nc.vector.tensor_copy(
                    vp[h][:, tb * (HD + 1):tb * (HD + 1) + HD], pv[:])
                nc.vector.tensor_copy(
                    vp[h][:, tb * (HD + 1) + HD:(tb + 1) * (HD + 1)],
                    ones_col[:])

        # ---- phase C: attention -------------------------------------
        for h in range(HPC):
            ro = (h % 2) * HD
            qh = qt[h // 2][ro:ro + HD, :]
            kh = kt[h // 2][ro:ro + HD, :]
            for ic in range(NTC):
                isl = slice(ic * TC, (ic + 1) * TC)
                njb = 4 * (ic + 1)
                # A_h^T for this i-chunk (base-partition-0 staging tile)
                pa = sp.tile([12, TC], F32, name="pa", tag="mm", bufs=4)
                nc.tensor.matmul(pa[:], bTt[:, 12 * h:12 * h + 12],
                                 uT[:, isl], start=True, stop=True)
                at_s = wp.tile([12, TC], SC, name="at_s", tag="at", bufs=2)
                nc.scalar.copy(at_s[:], pa[:])
                po = sp.tile([HD + 1, TC], F32, name="po", tag="po", bufs=2)
                for jb in range(njb):
                    jsl = slice(jb * JB, (jb + 1) * JB)
                    pb = sp.tile([128, TC], F32, name="pb", tag="mm", bufs=4)
                    nc.tensor.matmul(pb[:], uT[:, jsl], at_s[:],
                                     start=True, stop=True)
                    pr = sp.tile([128, TC], F32, name="pr", tag="mm", bufs=4)
                    nc.tensor.matmul(pr[:], kh[:, jsl], qh[:, isl],
                                     start=True, stop=True)
                    t1 = wp.tile([128, TC], SC, name="t1", tag="t1", bufs=3)
                    nc.scalar.activation(t1[:], pr[:], Act.Tanh, scale=inv_beta)
                    nc.vector.scalar_tensor_tensor(
                        t1[:], t1[:], alpha, pb[:], op0=Alu.mult, op1=Alu.add)
                    ee = wp.tile([128, TC], SC, name="ee", tag="ee", bufs=3)
                    nc.scalar.activation(ee[:], t1[:], Act.Exp)
                    off = jb * JB - ic * TC
                    if off >= 0:
                        # keep [p, f] where f >= p + off, else 0
                        nc.gpsimd.affine_select(
                            out=ee[:], in_=ee[:], compare_op=Alu.is_ge,
                            fill=0.0, base=-off, channel_multiplier=-1,
                            pattern=[[1, TC]])
                    nc.tensor.matmul(
                        po[:], vp[h][:, jb * (HD + 1):(jb + 1) * (HD + 1)],
                        ee[:], start=(jb == 0), stop=(jb == njb - 1))
                # normalize rows 0..63 by 1/row64: broadcast the denominator
                # down 64 partitions via a rank-1 matmul, then reciprocal on
                # all 64 lanes (a 1-partition reciprocal is ~13x slower).
                dn = wp.tile([1, TC], F32R, name="dn", tag="dn", bufs=2)
                nc.scalar.copy(dn[:], po[HD:HD + 1, :])
                prb = sp.tile([HD, TC], F32, name="prb", tag="aux", bufs=2)
                nc.tensor.matmul(prb[:], ones64[:], dn[:], start=True, stop=True)
                rb = wp.tile([HD, TC], F32, name="rb", tag="rb", bufs=2)
                nc.vector.reciprocal(rb[:], prb[:])
                nc.vector.tensor_mul(ao[h // 2][ro:ro + HD, isl],
                                     po[0:HD, :], rb[:])

        # ---- phase D: output projection (transposed partial) --------
        for ec in range(D // 128):
            esl = slice(ec * 128, (ec + 1) * 128)
            for t4 in range(NTC):
                sl = slice(t4 * TC, (t4 + 1) * TC)
                pf = sp.tile([128, TC], F32, name="pf", tag="po", bufs=2)
                nc.tensor.matmul(pf[:], owt[0][:, esl], ao[0][:, sl],
                                 start=True, stop=False)
                nc.tensor.matmul(pf[:], owt[1][:, esl], ao[1][:, sl],
                                 start=False, stop=True)
                fo = wp.tile([128, TC], F32, name="fo", tag="fo", bufs=3)
                nc.scalar.copy(fo[:], pf[:])
                nc.sync.dma_start(out=dr["poutT"][esl, sl], in_=fo[:])


def _build(alpha, inv_beta):
    SC = BF16 if USE_BF16 else F32R
    nc = bacc.Bacc("TRN2", debug=False)
    dr = {}
    dr["xT"] = nc.dram_tensor("xT", [D, T], SC, kind="ExternalInput").ap()
    dr["wqkvT"] = nc.dram_tensor(
        "wqkvT", [D, 3 * HPC * HD], SC, kind="ExternalInput").ap()
    dr["whqT"] = nc.dram_tensor("whqT", [D, 12], SC, kind="ExternalInput").ap()
    dr["bT"] = nc.dram_tensor("bT", [12, 12 * HPC], SC, kind="ExternalInput").ap()
    dr["owT"] = nc.dram_tensor(
        "owT", [HPC * HD, D], F32R, kind="ExternalInput").ap()
    dr["ident"] = nc.dram_tensor("ident", [128, 128], SC, kind="ExternalInput").ap()
    dr["ones64"] = nc.dram_tensor("ones64", [1, 64], F32R, kind="ExternalInput").ap()
    dr["poutT"] = nc.dram_tensor("poutT", [D, T], F32, kind="ExternalOutput").ap()
    with tile.TileContext(nc) as tc_:
        _emit(nc, tc_, dr, alpha, inv_beta)
    nc.compile()
    return nc


def _sigmoid(v):
    return 1.0 / (1.0 + np.exp(-v))


def _round_f32r(a):
    """Round fp32 -> fp32r bit pattern (11-bit mantissa, rte)."""
    u = np.ascontiguousarray(a, np.float32).view(np.uint32)
    r = (u + 0x7FF + ((u >> 12) & 1)) & np.uint32(0xFFFFF000)
    return r.view(np.float32)


def _sc_cast(a):
    """Cast an fp32 array to the score-chain wire dtype."""
    a = np.ascontiguousarray(a, np.float32)
    if USE_BF16:
        return a.astype(ml_dtypes.bfloat16)
    return _round_f32r(a)


def _host_prep(x, qkv_w, out_w, hex_w, hamming_lambda_logit, q6_w,
               transforms, transform_weights, scale_logit, sips_alpha,
               sips_beta):
    """Build the per-core input maps (all host work is slicing/transposes)."""
    x = np.asarray(x, np.float32)
    qkv_w = np.asarray(qkv_w, np.float32)
    out_w = np.asarray(out_w, np.float32)
    hex_w = np.asarray(hex_w, np.float32)
    q6_w = np.asarray(q6_w, np.float32)
    transforms = np.asarray(transforms, np.float32)
    transform_weights = np.asarray(transform_weights, np.float32)

    lam = float(_sigmoid(np.float32(hamming_lambda_logit)))
    scale2 = float(_sigmoid(np.float32(scale_logit))) * 2.0
    alpha = float(np.asarray(sips_alpha).reshape(-1)[0])
    inv_beta = 1.0 / float(np.asarray(sips_beta).reshape(-1)[0])

    tw = np.asarray(transform_weights, np.float64) / TEMP
    w = np.exp(tw - tw.max(-1, keepdims=True))
    w = (w / w.sum(-1, keepdims=True)).astype(np.float32)      # (H, NT)
    Mh = np.einsum("ht,tde->hde", w, transforms)               # (H, 6, 6)

    whqT = _sc_cast(np.vstack([hex_w, q6_w]).T)                # (D, 12)
    ident = _sc_cast(np.eye(128, dtype=np.float32))
    ones64 = np.ones((1, HD), np.float32)
    bigB = np.zeros((H, 12, 12), np.float32)
    for h in range(H):
        bigB[h, :6, :6] = (lam / 2.0) * np.eye(6, dtype=np.float32)
        bigB[h, 6:, 6:] = (scale2 / 6.0) * Mh[h]

    in_maps = []
    for core in range(NCORES):
        b = core // CPB
        heads = [(core % CPB) * HPC + k for k in range(HPC)]
        rows = []
        for part in range(3):
            for h in heads:
                rows.extend(range(part * D + h * HD, part * D + (h + 1) * HD))
        wqkvT = _sc_cast(qkv_w[rows, :].T)                      # (D, 768)
        cols = []
        for h in heads:
            cols.extend(range(h * HD, (h + 1) * HD))
        owT = _round_f32r(out_w[:, cols].T)                     # (256, D)
        bT = np.concatenate([bigB[h].T for h in heads], axis=1)  # (12, 48)
        in_maps.append({
            "xT": _sc_cast(x[b].T),
            "wqkvT": wqkvT,
            "whqT": whqT,
            "bT": _sc_cast(bT),
            "owT": owT,
            "ident": ident,
            "ones64": ones64,
        })
    return in_maps, alpha, inv_beta


_CACHE = {}
LAST_RESULT = None


def kernel(**inputs):
    global LAST_RESULT
    in_maps, alpha, inv_beta = _host_prep(**inputs)
    key = (round(alpha, 9), round(inv_beta, 9), USE_BF16)
    if key not in _CACHE:
        _CACHE[key] = _build(alpha, inv_beta)
    nc = _CACHE[key]
    res = run_bass_kernel_spmd(nc, in_maps, list(range(NCORES)))
    LAST_RESULT = res
    out = np.zeros((B, T, D), np.float32)
    for b in range(B):
        acc = np.zeros((D, T), np.float32)
        for core in range(b * CPB, (b + 1) * CPB):
            acc += res.results[core]["poutT"]
        out[b] = acc.T
    return out
